# revision 1
# baseline (speedup 1.0000x reference)
"""GCN autoencoder (6x gcn_layer) on 8 TRN2 NeuronCores.

Strategy:
  - Rows of adj_ (and X) sharded across 8 cores; weights replicated.
  - Device layout: adjT shard [8192, 1024] bf16 kept SBUF-resident for all
    6 layers (avoids 6x HBM re-read of the 256MB adjacency).
  - Per layer l:  H_local = z_shard @ W_l   (local rows, small matmul)
                  AllGather(H_local) -> H_full [8192, d_out]
                  zT_shard = relu(adj_shard @ H_full)^T   (PE, bf16, fp32 accum)
    zT_shard [d_out, 1024] stays in SBUF as the stationary operand of the
    next layer's XW matmul (no transposes needed anywhere on device).
  - Host does the (free) sharding / transposition / dtype casts and the
    final gather + transpose.
"""

import sys

import numpy as np

if "/opt/trn_rl_repo" not in sys.path:
    sys.path.insert(0, "/opt/trn_rl_repo")

import ml_dtypes

import concourse.bacc as bacc
import concourse.bass as bass
import concourse.tile as tile
from concourse import mybir
from concourse.bass_utils import run_bass_kernel_spmd

N = 8192
D_IN = 512
NCORES = 8
R = N // NCORES  # 1024 rows per core
DIMS = [(512, 256), (256, 256), (256, 128), (128, 256), (256, 256), (256, 512)]

BF16 = mybir.dt.bfloat16
F32 = mybir.dt.float32
NP_BF16 = ml_dtypes.bfloat16

_CACHED = {}


def _build():
    nc = bacc.Bacc(
        "TRN2",
        target_bir_lowering=False,
        debug=False,
        enable_asserts=False,
        num_devices=NCORES,
    )

    adjT = nc.dram_tensor("adjT", [N, R], BF16, kind="ExternalInput")
    xT = nc.dram_tensor("xT", [D_IN, R], BF16, kind="ExternalInput")
    w_dram = [
        nc.dram_tensor(f"W{i + 1}", list(DIMS[i]), BF16, kind="ExternalInput")
        for i in range(6)
    ]
    outT = nc.dram_tensor("outT", [DIMS[-1][1], R], F32, kind="ExternalOutput")

    KO = N // 128  # 64 k-chunks over the gather dim
    RT = R // 128  # 8 row tiles per core

    with tile.TileContext(nc) as tc:
        with (
            tc.tile_pool(name="adjp", bufs=1) as adjp,
            tc.tile_pool(name="wp", bufs=1) as wp,
            tc.tile_pool(name="zt", bufs=8) as ztp,
            tc.tile_pool(name="hp", bufs=1) as hp,
            tc.tile_pool(name="hstage", bufs=4) as hstage,
            tc.tile_pool(name="ostage", bufs=4) as ostage,
            tc.tile_pool(name="psz", bufs=4, space="PSUM") as psz,
            tc.tile_pool(name="psh", bufs=2, space="PSUM") as psh,
            tc.tile_pool(name="dram", bufs=1, space="DRAM") as dram,
        ):
            # ---- resident adjacency: [128, 64, 1024] bf16 (16 MB) ----
            adj_sb = adjp.tile([128, KO, R], BF16)
            adjT_r = adjT.ap().rearrange("(ko p) r -> p ko r", p=128)
            for j in range(0, KO, 4):
                nc.sync.dma_start(adj_sb[:, j : j + 4, :], adjT_r[:, j : j + 4, :])

            # ---- resident weights: [128, d_in/128, d_out] bf16 each ----
            w_sb = []
            for i, (di, do) in enumerate(DIMS):
                kx = di // 128
                w_t = wp.tile([128, kx, do], BF16, name=f"w{i}_sb")
                nc.sync.dma_start(
                    w_t[:], w_dram[i].ap().rearrange("(kx p) n -> p kx n", p=128)
                )
                w_sb.append(w_t)

            # ---- layer-0 activations: xT as 4 zT tiles [128, 1024] ----
            zt_tiles = []
            for m in range(D_IN // 128):
                z_t = ztp.tile([128, R], BF16, tag="zt", name=f"z0_{m}")
                nc.sync.dma_start(z_t[:], xT[m * 128 : (m + 1) * 128, :])
                zt_tiles.append(z_t)

            for li, (di, do) in enumerate(DIMS):
                last = li == len(DIMS) - 1
                kxn = di // 128
                # column chunks of <=256 for the gather (keeps H_sb <= 4MB)
                chunks = [(c, min(256, do - c)) for c in range(0, do, 256)]

                # ---- XW: H_local[r*128:(r+1)*128, :] = z_shard @ W ----
                h_bounces = [
                    dram.tile([R, dc], BF16, tag=f"hb{li}_{c}", name=f"hb{li}_{c}")
                    for (c, dc) in chunks
                ]
                for r in range(RT):
                    ps_h = psh.tile([128, do], F32, tag="psh")
                    for kx in range(kxn):
                        nc.tensor.matmul(
                            ps_h[:],
                            zt_tiles[kx][:, r * 128 : (r + 1) * 128],
                            w_sb[li][:, kx, :],
                            start=(kx == 0),
                            stop=(kx == kxn - 1),
                        )
                    for ci, (c, dc) in enumerate(chunks):
                        h_st = hstage.tile([128, dc], BF16, tag="hst")
                        nc.vector.tensor_copy(h_st[:], ps_h[:, c : c + dc])
                        nc.sync.dma_start(
                            h_bounces[ci][r * 128 : (r + 1) * 128, :], h_st[:]
                        )

                new_zt = [None] * (do // 128)
                for ci, (c, dc) in enumerate(chunks):
                    # ---- AllGather this column chunk ----
                    h_gath = dram.tile(
                        [N, dc],
                        BF16,
                        addr_space="Shared",
                        tag=f"hg{li}_{c}",
                        name=f"hg{li}_{c}",
                    )
                    nc.gpsimd.collective_compute(
                        "AllGather",
                        mybir.AluOpType.bypass,
                        ins=[h_bounces[ci][:].opt()],
                        outs=[h_gath[:].opt()],
                        replica_groups=[list(range(NCORES))],
                    )
                    # ---- load gathered H chunk: [128, 64, dc] ----
                    h_sb = hp.tile([128, KO, dc], BF16, tag="hsb", name=f"hsb{li}_{c}")
                    h_gr = h_gath.rearrange("(ko p) d -> p ko d", p=128)
                    for j in range(0, KO, 16):
                        nc.sync.dma_start(
                            h_sb[:, j : j + 16, :], h_gr[:, j : j + 16, :]
                        )

                    # ---- adj-mm: zT[m] = relu(adj @ H)^T ----
                    for ml in range(dc // 128):
                        mg = c // 128 + ml
                        if not last:
                            z_t = ztp.tile(
                                [128, R], BF16, tag="zt", name=f"z{li + 1}_{mg}"
                            )
                            new_zt[mg] = z_t
                        for n in range(R // 512):
                            ps_z = psz.tile([128, 512], F32, tag="psz")
                            for k in range(KO):
                                nc.tensor.matmul(
                                    ps_z[:],
                                    h_sb[:, k, ml * 128 : (ml + 1) * 128],
                                    adj_sb[:, k, n * 512 : (n + 1) * 512],
                                    start=(k == 0),
                                    stop=(k == KO - 1),
                                )
                            if last:
                                o_st = ostage.tile([128, 512], F32, tag="ost")
                                nc.scalar.activation(
                                    o_st[:], ps_z[:], mybir.ActivationFunctionType.Relu
                                )
                                nc.sync.dma_start(
                                    outT[
                                        mg * 128 : (mg + 1) * 128,
                                        n * 512 : (n + 1) * 512,
                                    ],
                                    o_st[:],
                                )
                            else:
                                nc.scalar.activation(
                                    new_zt[mg][:, n * 512 : (n + 1) * 512],
                                    ps_z[:],
                                    mybir.ActivationFunctionType.Relu,
                                )
                if not last:
                    zt_tiles = new_zt

    nc.compile()
    return nc


def kernel(**inputs):
    X = np.asarray(inputs["X"], dtype=np.float32)
    adj = np.asarray(inputs["adj_"], dtype=np.float32)

    if "nc" not in _CACHED:
        _CACHED["nc"] = _build()
    nc = _CACHED["nc"]

    in_maps = []
    for i in range(NCORES):
        rows = slice(i * R, (i + 1) * R)
        m = {
            "adjT": np.ascontiguousarray(adj[rows, :].T).astype(NP_BF16),
            "xT": np.ascontiguousarray(X[rows, :].T).astype(NP_BF16),
        }
        for j in range(6):
            m[f"W{j + 1}"] = np.asarray(inputs[f"W{j + 1}"], np.float32).astype(NP_BF16)
        in_maps.append(m)

    res = run_bass_kernel_spmd(nc, in_maps, core_ids=list(range(NCORES)))
    out = np.concatenate(
        [np.asarray(r["outT"], dtype=np.float32).T for r in res.results], axis=0
    )
    return out


# revision 4
# speedup vs baseline: 1.1932x; 1.1932x over previous
"""GCN autoencoder (6x gcn_layer) on 8 TRN2 NeuronCores.

Strategy (v2):
  - Rows of adj_/X sharded across 8 cores; weights replicated.
  - All device tensors bf16 (fp32 PSUM accumulation); host does the free
    sharding / transposes / casts and the final gather+transpose.
  - Per layer l (adj-mm produces zT = (adj_shard @ H)^T so the next XW
    needs no transposes):
      phase n=0:  mm over rows 0:512   -> zT[:, 0:512]
                  then XW(l+1) for those rows -> bounce -> AllGather(H n0)
                  (flies while phase n=1 computes)
      phase n=1:  mm over rows 512:1024 -> XW(l+1) -> AllGather(H n1)
                  (lands during layer l+1's first k-chunks)
    Layer l+1 accumulates its 64 k-chunks in arrival order (n0-sourced
    chunks first), so the n1 gather is hidden under its matmul stream.
  - adj columns 0:512 SBUF-resident; columns 512:1024 streamed per layer.
  - Layer 1's H1 = X @ W1 is computed fully on every core from the
    (replicated, free) input X -> no collective before the first adj-mm.
"""

import sys

import numpy as np

if "/opt/trn_rl_repo" not in sys.path:
    sys.path.insert(0, "/opt/trn_rl_repo")

import ml_dtypes

import concourse.bacc as bacc
import concourse.tile as tile
from concourse import mybir
from concourse.bass_utils import run_bass_kernel_spmd

N = 8192
D_IN = 512
NCORES = 8
R = N // NCORES  # 1024 rows per core
DIMS = [(512, 256), (256, 256), (256, 128), (128, 256), (256, 256), (256, 512)]

BF16 = mybir.dt.bfloat16
F32 = mybir.dt.float32
NP_BF16 = ml_dtypes.bfloat16
RELU = mybir.ActivationFunctionType.Relu

KO = N // 128  # 64 k-chunks over the gather dim
RT = R // 128  # 8 local row tiles
NPH = 2  # row phases per layer (512 rows each)
PH = R // NPH  # 512

_CACHED = {}


def _build():
    nc = bacc.Bacc(
        "TRN2",
        target_bir_lowering=False,
        debug=False,
        enable_asserts=False,
        num_devices=NCORES,
    )

    adjT = nc.dram_tensor("adjT", [N, R], BF16, kind="ExternalInput")
    xT = nc.dram_tensor("xT", [D_IN, N], BF16, kind="ExternalInput")
    w_dram = [
        nc.dram_tensor(f"W{i + 1}", list(DIMS[i]), BF16, kind="ExternalInput")
        for i in range(6)
    ]
    outT = nc.dram_tensor("outT", [DIMS[-1][1], R], F32, kind="ExternalOutput")

    adjT_r = adjT.ap().rearrange("(ko p) r -> p ko r", p=128)
    xT_r = xT.ap().rearrange("(kx p) c -> p kx c", p=128)

    with tile.TileContext(nc) as tc:
        with (
            tc.tile_pool(name="adjres", bufs=1) as adjres_p,
            tc.tile_pool(name="adjstr", bufs=8) as adjstr_p,
            tc.tile_pool(name="wp", bufs=1) as wp,
            tc.tile_pool(name="xtp", bufs=3) as xtp,
            tc.tile_pool(name="ztp", bufs=12) as ztp,
            tc.tile_pool(name="hp", bufs=3) as hp,
            tc.tile_pool(name="hstage", bufs=4) as hstage,
            tc.tile_pool(name="ostage", bufs=3) as ostage,
            tc.tile_pool(name="psz", bufs=4, space="PSUM") as psz,
            tc.tile_pool(name="psh", bufs=3, space="PSUM") as psh,
            tc.tile_pool(name="dram", bufs=1, space="DRAM") as dram,
        ):
            # ---- resident weights ----
            w_sb = []
            for i, (di, do) in enumerate(DIMS):
                w_t = wp.tile([128, di // 128, do], BF16, name=f"w{i}_sb")
                nc.sync.dma_start(
                    w_t[:], w_dram[i].ap().rearrange("(kx p) n -> p kx n", p=128)
                )
                w_sb.append(w_t)

            # ---- resident adj columns 0:512 : [128, 64, 512] bf16 (8MB) ----
            adj_res = adjres_p.tile([128, KO, PH], BF16)
            for j in range(0, KO, 8):
                nc.sync.dma_start(
                    adj_res[:, j : j + 8, :], adjT_r[:, j : j + 8, 0:PH]
                )

            def adj_mov(g, n):
                """moving operand for k-chunk g, row-phase n (as SBUF AP)."""
                if n == 0:
                    return adj_res[:, g, :]
                t = adjstr_p.tile([128, PH], BF16, tag="adjs", name=f"as{g}")
                nc.sync.dma_start(t[:], adjT_r[:, g, PH:R])
                return t[:]

            # ---- layer 1: H1 = X @ W1 computed fully on every core ----
            h_cur = hp.tile([128, KO, DIMS[0][1]], BF16, tag="h", name="h1")
            for g0 in range(0, KO, 2):
                xt_t = xtp.tile([128, D_IN // 128, 256], BF16, tag="xt")
                nc.sync.dma_start(xt_t[:], xT_r[:, :, g0 * 128 : g0 * 128 + 256])
                for g in (g0, g0 + 1):
                    ps_h = psh.tile([128, DIMS[0][1]], F32, tag="psh")
                    for kx in range(D_IN // 128):
                        c = (g - g0) * 128
                        nc.tensor.matmul(
                            ps_h[:],
                            xt_t[:, kx, c : c + 128],
                            w_sb[0][:, kx, :],
                            start=(kx == 0),
                            stop=(kx == D_IN // 128 - 1),
                        )
                    nc.vector.tensor_copy(h_cur[:, g, :], ps_h[:])

            # k-chunk consumption order for layer l's accumulation:
            # layer 1: production order (g ascending).
            # layers >=2: chunks fed by the producer's n0 phase first
            # (chunk ids delivered by producer phase n: {c*8 + n*4 + j, j<4})
            k_order_l1 = list(range(KO))
            wave = [
                [c * RT + n * (RT // NPH) + j
                 for c in range(NCORES) for j in range(RT // NPH)]
                for n in range(NPH)
            ]
            k_order_gather = wave[0] + wave[1]

            for li, (di, do) in enumerate(DIMS):
                last = li == len(DIMS) - 1
                mt = do // 128
                korder = k_order_l1 if li == 0 else k_order_gather

                # next layer setup
                if not last:
                    di2, do2 = DIMS[li + 1]
                    kxn2 = di2 // 128  # == mt
                    # H_{l+2... } buffer(s) for layer li+1, filled via AG
                    if do2 <= 256:
                        h_next = [hp.tile([128, KO, do2], BF16, tag="h",
                                          name=f"h{li + 2}")]
                        nsplit = [(0, do2)]
                    else:  # layer 6: split columns into two 256 buffers
                        h_next = [
                            hp.tile([128, KO, 256], BF16, tag="h",
                                    name=f"h{li + 2}a"),
                            hp.tile([128, KO, 256], BF16, tag="h",
                                    name=f"h{li + 2}b"),
                        ]
                        nsplit = [(0, 256), (256, 256)]

                def h_lhsT(m, g):
                    if isinstance(h_cur, list):
                        return h_cur[m // 2][:, g, (m % 2) * 128 : (m % 2) * 128 + 128]
                    return h_cur[:, g, m * 128 : (m + 1) * 128]

                for n in range(NPH):
                    # ---- adj-mm phase n: zT[:, n*512:(n+1)*512] ----
                    # k-outer so each streamed adj chunk is fetched once and
                    # shared by all m tiles; mt psum banks accumulate together.
                    ps_zs = [psz.tile([128, PH], F32, tag="psz", name=f"psz{m}")
                             for m in range(mt)]
                    for ki, g in enumerate(korder):
                        mov = adj_mov(g, n)
                        for m in range(mt):
                            nc.tensor.matmul(
                                ps_zs[m][:],
                                h_lhsT(m, g),
                                mov,
                                start=(ki == 0),
                                stop=(ki == KO - 1),
                            )
                    zt_n = []
                    for m in range(mt):
                        if last:
                            o_st = ostage.tile([128, PH], F32, tag="ost")
                            nc.scalar.activation(o_st[:], ps_zs[m][:], RELU)
                            nc.sync.dma_start(
                                outT[m * 128 : (m + 1) * 128, n * PH : (n + 1) * PH],
                                o_st[:],
                            )
                            zt_n.append(None)
                        else:
                            z_t = ztp.tile([128, PH], BF16, tag="zt",
                                           name=f"z{li + 1}_{m}_{n}")
                            nc.scalar.activation(z_t[:], ps_zs[m][:], RELU)
                            zt_n.append(z_t)

                    if last:
                        continue

                    # ---- XW(l+1) for this phase's rows + AG ----
                    bounces = [
                        dram.tile([PH, dc], BF16, tag=f"hb{li}_{n}_{ci}",
                                  name=f"hb{li}_{n}_{ci}")
                        for ci, (c0, dc) in enumerate(nsplit)
                    ]
                    for j in range(RT // NPH):  # 4 row tiles in this phase
                        ps_h = psh.tile([128, do2], F32, tag="psh")
                        for kx in range(kxn2):
                            nc.tensor.matmul(
                                ps_h[:],
                                zt_n[kx][:, j * 128 : (j + 1) * 128],
                                w_sb[li + 1][:, kx, :],
                                start=(kx == 0),
                                stop=(kx == kxn2 - 1),
                            )
                        for ci, (c0, dc) in enumerate(nsplit):
                            h_st = hstage.tile([128, dc], BF16, tag="hst")
                            nc.vector.tensor_copy(h_st[:], ps_h[:, c0 : c0 + dc])
                            nc.sync.dma_start(
                                bounces[ci][j * 128 : (j + 1) * 128, :], h_st[:]
                            )
                    for ci, (c0, dc) in enumerate(nsplit):
                        gath = dram.tile(
                            [NCORES * PH, dc], BF16, addr_space="Shared",
                            tag=f"hg{li}_{n}_{ci}", name=f"hg{li}_{n}_{ci}",
                        )
                        nc.gpsimd.collective_compute(
                            "AllGather",
                            mybir.AluOpType.bypass,
                            ins=[bounces[ci][:].opt()],
                            outs=[gath[:].opt()],
                            replica_groups=[list(range(NCORES))],
                        )
                        g_r = gath.rearrange("(q p) d -> p q d", p=128)
                        half = RT // NPH  # 4
                        for c in range(NCORES):
                            nc.sync.dma_start(
                                h_next[ci][:, c * RT + n * half : c * RT + n * half + half, :],
                                g_r[:, c * half : (c + 1) * half, :],
                            )

                if not last:
                    h_cur = h_next if len(h_next) > 1 else h_next[0]

    nc.compile()
    return nc


def kernel(**inputs):
    X = np.asarray(inputs["X"], dtype=np.float32)
    adj = np.asarray(inputs["adj_"], dtype=np.float32)

    if "nc" not in _CACHED:
        _CACHED["nc"] = _build()
    nc = _CACHED["nc"]

    xT_full = np.ascontiguousarray(X.T).astype(NP_BF16)
    ws = [np.asarray(inputs[f"W{j + 1}"], np.float32).astype(NP_BF16) for j in range(6)]
    in_maps = []
    for i in range(NCORES):
        rows = slice(i * R, (i + 1) * R)
        m = {
            "adjT": np.ascontiguousarray(adj[rows, :].T).astype(NP_BF16),
            "xT": xT_full,
        }
        for j in range(6):
            m[f"W{j + 1}"] = ws[j]
        in_maps.append(m)

    res = run_bass_kernel_spmd(nc, in_maps, core_ids=list(range(NCORES)))
    out = np.concatenate(
        [np.asarray(r["outT"], dtype=np.float32).T for r in res.results], axis=0
    )
    return out


# revision 10
# speedup vs baseline: 1.2406x; 1.0397x over previous
"""GCN autoencoder (6x gcn_layer) on 8 TRN2 NeuronCores.

Strategy (v2):
  - Rows of adj_/X sharded across 8 cores; weights replicated.
  - All device tensors bf16 (fp32 PSUM accumulation); host does the free
    sharding / transposes / casts and the final gather+transpose.
  - Per layer l (adj-mm produces zT = (adj_shard @ H)^T so the next XW
    needs no transposes):
      phase n=0:  mm over rows 0:512   -> zT[:, 0:512]
                  then XW(l+1) for those rows -> bounce -> AllGather(H n0)
                  (flies while phase n=1 computes)
      phase n=1:  mm over rows 512:1024 -> XW(l+1) -> AllGather(H n1)
                  (lands during layer l+1's first k-chunks)
    Layer l+1 accumulates its 64 k-chunks in arrival order (n0-sourced
    chunks first), so the n1 gather is hidden under its matmul stream.
  - adj columns 0:512 SBUF-resident; columns 512:1024 streamed per layer.
  - Layer 1's H1 = X @ W1 is computed fully on every core from the
    (replicated, free) input X -> no collective before the first adj-mm.
"""

import sys

import numpy as np

if "/opt/trn_rl_repo" not in sys.path:
    sys.path.insert(0, "/opt/trn_rl_repo")

import ml_dtypes

import concourse.bacc as bacc
import concourse.tile as tile
from concourse import mybir
from concourse.bass_utils import run_bass_kernel_spmd

N = 8192
D_IN = 512
NCORES = 8
R = N // NCORES  # 1024 rows per core
DIMS = [(512, 256), (256, 256), (256, 128), (128, 256), (256, 256), (256, 512)]

BF16 = mybir.dt.bfloat16
F32 = mybir.dt.float32
NP_BF16 = ml_dtypes.bfloat16
RELU = mybir.ActivationFunctionType.Relu

KO = N // 128  # 64 k-chunks over the gather dim
RT = R // 128  # 8 local row tiles
NPH = 2  # row phases per layer (512 rows each)
PH = R // NPH  # 512

_CACHED = {}


def _build():
    nc = bacc.Bacc(
        "TRN2",
        target_bir_lowering=False,
        debug=False,
        enable_asserts=False,
        num_devices=NCORES,
    )

    adjT = nc.dram_tensor("adjT", [N, R], BF16, kind="ExternalInput")
    xT = nc.dram_tensor("xT", [D_IN, N], BF16, kind="ExternalInput")
    w_dram = [
        nc.dram_tensor(f"W{i + 1}", list(DIMS[i]), BF16, kind="ExternalInput")
        for i in range(6)
    ]
    outT = nc.dram_tensor("outT", [DIMS[-1][1], R], F32, kind="ExternalOutput")

    adjT_r = adjT.ap().rearrange("(ko p) r -> p ko r", p=128)
    xT_r = xT.ap().rearrange("(kx p) c -> p kx c", p=128)

    with tile.TileContext(nc) as tc:
        with (
            tc.tile_pool(name="adjres", bufs=1) as adjres_p,
            tc.tile_pool(name="adjstr", bufs=3) as adjstr_p,
            tc.tile_pool(name="wp", bufs=1) as wp,
            tc.tile_pool(name="xtp", bufs=3) as xtp,
            tc.tile_pool(name="ztp", bufs=12) as ztp,
            tc.tile_pool(name="hp", bufs=3) as hp,
            tc.tile_pool(name="hstage", bufs=4) as hstage,
            tc.tile_pool(name="ostage", bufs=2) as ostage,
            tc.tile_pool(name="psz", bufs=5, space="PSUM") as psz,
            tc.tile_pool(name="psh", bufs=3, space="PSUM") as psh,
            tc.tile_pool(name="dram", bufs=1, space="DRAM") as dram,
        ):
            # ---- resident weights ----
            w_sb = []
            for i, (di, do) in enumerate(DIMS):
                w_t = wp.tile([128, di // 128, do], BF16, name=f"w{i}_sb")
                nc.sync.dma_start(
                    w_t[:], w_dram[i].ap().rearrange("(kx p) n -> p kx n", p=128)
                )
                w_sb.append(w_t)

            # ---- resident adj columns 0:512 : [128, 64, 512] bf16 (8MB) ----
            # (DMAs emitted inside the layer-1 section so the small xT/W
            #  loads win the queues at startup)
            adj_res = adjres_p.tile([128, KO, PH], BF16)

            adj_stream_cache = {}

            def adj_mov(g, n):
                """moving operand for k-chunk g, row-phase n (as SBUF AP)."""
                if n == 0:
                    return adj_res[:, g, :]
                grp = g // 4
                t = adj_stream_cache.get(grp)
                if t is None:
                    t = adjstr_p.tile([128, 4, PH], BF16, tag="adjs",
                                      name=f"as{grp}")
                    nc.sync.dma_start(t[:], adjT_r[:, grp * 4 : grp * 4 + 4, PH:R])
                    adj_stream_cache[grp] = t
                return t[:, g % 4, :]

            # ---- layer 1: H1 = X @ W1 computed fully on every core ----
            h_cur = hp.tile([128, KO, DIMS[0][1]], BF16, tag="h", name="h1")
            for g0 in range(0, KO, 2):
                xt_t = xtp.tile([128, D_IN // 128, 256], BF16, tag="xt")
                nc.sync.dma_start(xt_t[:], xT_r[:, :, g0 * 128 : g0 * 128 + 256])
                for g in (g0, g0 + 1):
                    ps_h = psh.tile([128, DIMS[0][1]], F32, tag="psh")
                    for kx in range(D_IN // 128):
                        c = (g - g0) * 128
                        nc.tensor.matmul(
                            ps_h[:],
                            xt_t[:, kx, c : c + 128],
                            w_sb[0][:, kx, :],
                            start=(kx == 0),
                            stop=(kx == D_IN // 128 - 1),
                        )
                    nc.vector.tensor_copy(h_cur[:, g, :], ps_h[:])

            # resident-adj load, emitted after the XW1 stream so the small
            # xT/W DMAs get the queues first; k-ordered to match consumption
            for j in range(0, KO, 4):
                nc.sync.dma_start(
                    adj_res[:, j : j + 4, :], adjT_r[:, j : j + 4, 0:PH]
                )

            # k-chunk consumption order for layer l's accumulation:
            # layer 1: production order (g ascending).
            # layers >=2: chunks fed by the producer's n0 phase first
            # (chunk ids delivered by producer phase n: {c*8 + n*4 + j, j<4})
            k_order_l1 = list(range(KO))
            wave = [
                [c * RT + n * (RT // NPH) + j
                 for c in range(NCORES) for j in range(RT // NPH)]
                for n in range(NPH)
            ]
            k_order_gather = wave[0] + wave[1]

            for li, (di, do) in enumerate(DIMS):
                last = li == len(DIMS) - 1
                mt = do // 128
                korder = k_order_l1 if li == 0 else k_order_gather

                # next layer setup
                if not last:
                    di2, do2 = DIMS[li + 1]
                    kxn2 = di2 // 128  # == mt
                    # H_{l+2... } buffer(s) for layer li+1, filled via AG
                    if do2 <= 256:
                        h_next = [hp.tile([128, KO, do2], BF16, tag="h",
                                          name=f"h{li + 2}")]
                        nsplit = [(0, do2)]
                    else:  # layer 6: split columns into two 256 buffers
                        h_next = [
                            hp.tile([128, KO, 256], BF16, tag="h",
                                    name=f"h{li + 2}a"),
                            hp.tile([128, KO, 256], BF16, tag="h",
                                    name=f"h{li + 2}b"),
                        ]
                        nsplit = [(0, 256), (256, 256)]

                def h_lhsT(m, g):
                    if isinstance(h_cur, list):
                        return h_cur[m // 2][:, g, (m % 2) * 128 : (m % 2) * 128 + 128]
                    return h_cur[:, g, m * 128 : (m + 1) * 128]

                for n in range(NPH):
                    # ---- adj-mm phase n: zT[:, n*512:(n+1)*512] ----
                    # k-outer so each streamed adj chunk is fetched once and
                    # shared by all m tiles; mt psum banks accumulate together.
                    adj_stream_cache.clear()
                    ps_zs = [psz.tile([128, PH], F32, tag="psz", name=f"psz{m}")
                             for m in range(mt)]
                    for ki, g in enumerate(korder):
                        mov = adj_mov(g, n)
                        for m in range(mt):
                            nc.tensor.matmul(
                                ps_zs[m][:],
                                h_lhsT(m, g),
                                mov,
                                start=(ki == 0),
                                stop=(ki == KO - 1),
                            )
                    zt_n = []
                    for m in range(mt):
                        if last:
                            o_st = ostage.tile([128, PH], F32, tag="ost")
                            nc.scalar.activation(o_st[:], ps_zs[m][:], RELU)
                            nc.sync.dma_start(
                                outT[m * 128 : (m + 1) * 128, n * PH : (n + 1) * PH],
                                o_st[:],
                            )
                            zt_n.append(None)
                        else:
                            z_t = ztp.tile([128, PH], BF16, tag="zt",
                                           name=f"z{li + 1}_{m}_{n}")
                            nc.scalar.activation(z_t[:], ps_zs[m][:], RELU)
                            zt_n.append(z_t)

                    if last:
                        continue

                    # ---- XW(l+1) for this phase's rows + AG ----
                    bounces = [
                        dram.tile([PH, dc], BF16, tag=f"hb{li}_{n}_{ci}",
                                  name=f"hb{li}_{n}_{ci}")
                        for ci, (c0, dc) in enumerate(nsplit)
                    ]
                    for j in range(RT // NPH):  # 4 row tiles in this phase
                        ps_h = psh.tile([128, do2], F32, tag="psh")
                        for kx in range(kxn2):
                            nc.tensor.matmul(
                                ps_h[:],
                                zt_n[kx][:, j * 128 : (j + 1) * 128],
                                w_sb[li + 1][:, kx, :],
                                start=(kx == 0),
                                stop=(kx == kxn2 - 1),
                            )
                        for ci, (c0, dc) in enumerate(nsplit):
                            h_st = hstage.tile([128, dc], BF16, tag="hst")
                            nc.vector.tensor_copy(h_st[:], ps_h[:, c0 : c0 + dc])
                            nc.sync.dma_start(
                                bounces[ci][j * 128 : (j + 1) * 128, :], h_st[:]
                            )
                    for ci, (c0, dc) in enumerate(nsplit):
                        gath = dram.tile(
                            [NCORES * PH, dc], BF16, addr_space="Shared",
                            tag=f"hg{li}_{n}_{ci}", name=f"hg{li}_{n}_{ci}",
                        )
                        nc.gpsimd.collective_compute(
                            "AllGather",
                            mybir.AluOpType.bypass,
                            ins=[bounces[ci][:].opt()],
                            outs=[gath[:].opt()],
                            replica_groups=[list(range(NCORES))],
                        )
                        g_r = gath.rearrange("(q p) d -> p q d", p=128)
                        half = RT // NPH  # 4
                        for c in range(NCORES):
                            nc.sync.dma_start(
                                h_next[ci][:, c * RT + n * half : c * RT + n * half + half, :],
                                g_r[:, c * half : (c + 1) * half, :],
                            )

                if not last:
                    h_cur = h_next if len(h_next) > 1 else h_next[0]

    nc.compile()
    return nc


def kernel(**inputs):
    X = np.asarray(inputs["X"], dtype=np.float32)
    adj = np.asarray(inputs["adj_"], dtype=np.float32)

    if "nc" not in _CACHED:
        _CACHED["nc"] = _build()
    nc = _CACHED["nc"]

    xT_full = np.ascontiguousarray(X.T).astype(NP_BF16)
    ws = [np.asarray(inputs[f"W{j + 1}"], np.float32).astype(NP_BF16) for j in range(6)]
    in_maps = []
    for i in range(NCORES):
        rows = slice(i * R, (i + 1) * R)
        m = {
            "adjT": np.ascontiguousarray(adj[rows, :].T).astype(NP_BF16),
            "xT": xT_full,
        }
        for j in range(6):
            m[f"W{j + 1}"] = ws[j]
        in_maps.append(m)

    res = run_bass_kernel_spmd(nc, in_maps, core_ids=list(range(NCORES)))
    out = np.concatenate(
        [np.asarray(r["outT"], dtype=np.float32).T for r in res.results], axis=0
    )
    return out
